# revision 73
# baseline (speedup 1.0000x reference)
"""Cached self-attention (QK-RMSNorm + RoPE + extend-cache MHA + out-proj),
tensor-parallel over heads across 8 trn2 NeuronCores.

Sharding: Wq/Wk/Wv column-sharded (3 heads = 384 dims per core), Wo
row-sharded; each core owns its slice of the KV cache. The QK RMSNorm is over
the full 3072-dim vector, so per-core partial sum-of-squares are AllReduced
(tiny [128,8] tensor) and the rstd scale is applied after the reduce lands;
the output projection produces per-core partial sums over the full model dim
which the host reduces.

Precision: all streamed operands (x, weights, KV cache, probs, attention
output, rope tables) are bf16; every contraction accumulates fp32 in PSUM.
The softmax denominator is accumulated in four rotating bf16 SBUF tiles on
the Vector engine (bounded-depth sums keep the bf16 rounding ~0.2%), reduced
across partitions on GpSimd, and folded into the PSUM->SBUF normalize.
Measured end-to-end relative error ~5.3e-3 of absmax vs the fp32 reference.

Schedule highlights (284us -> 187us on the Tile cost model):
  - x and Wq stream in 4-chunk groups (first transfers halved) so the first
    projection matmul issues ~3.5us in instead of after the full 6MB x load;
    the other consts ride in later DMA slack.
  - Per-projection epilogues order bias-adds (frees PSUM banks for the next
    projection) then ssq partials (gates the AllReduce) then gain+RoPE; the
    ssq AllReduce overlaps the V projection and the first K/V chunk
    prefetches. The V bias is folded into the projection itself as a rank-1
    ones x bias matmul so its epilogue is a plain ACT copy.
  - Head transposes batch 4 row-chunks per PSUM bank -> one [128,512] DVE
    copy per head, keeping the projection->attention PSUM handoff short.
  - Scalar-engine activation-table loads (1.3us each) are pinned off the
    critical path: a dummy Sqrt at startup, a dummy Exp (gated on rstd)
    right after the Sqrt so the first score exp hits a warm table.
  - Attention runs on 6-s-tile score blocks ([128,1536] PSUM, 2-deep ring):
    exp amortizes its fixed cost, scores/AV software-pipeline 2 blocks deep,
    and the pipeline SPANS head boundaries (each pend entry carries its
    head's PSUM half + start/stop flags; the denominator finalize emits when
    the head's last AV group flushes).
  - Softmax denominators never touch the Tensor engine (was ~42us of M=1
    ones-matmuls): exp blocks accumulate on DVE with the accumulator ring
    arranged so only one cross-fold remains after the last block, then
    partition_all_reduce on GpSimd returns per-query sums already broadcast.
  - b0's output projection trickles one 512-col chunk per 2 blocks into b1's
    (ACT-bound) attention stream, using a dedicated PSUM bank so it can't
    flip the score ring parity; b1's runs at the end on the freed score ring
    with PE "heater" matmuls over the final normalize window to hold the
    2.4GHz clock state.
"""

import ml_dtypes
import numpy as np

import concourse.bass as bass
import concourse.bass_isa as bass_isa
import concourse.mybir as mybir
import concourse.tile as tile
from concourse import bacc
from concourse.bass import ts
from concourse.bass_utils import run_bass_kernel_spmd
from concourse.masks import make_identity

F32 = mybir.dt.float32
BF16 = mybir.dt.bfloat16
AF = mybir.ActivationFunctionType
OP = mybir.AluOpType

B = 2
S_NEW = 256
DIM = 3072
NUM_HEADS = 24
HD = 128
EPS = 1e-6
NCORES = 8
HL = NUM_HEADS // NCORES  # heads per core: 3
CD = HL * HD  # per-core head dims: 384
R = B * S_NEW  # 512 query rows, r = b*256 + s
RC = R // 128  # 4 row chunks
NI = DIM // 128  # 24 contraction chunks
G = 4  # contraction chunks per streamed weight group
NG = NI // G
SCALE = 1.0 / np.sqrt(HD)


def build(s_cached: int, s_chunk: int, collective: bool = True):
    """Build the per-core SPMD module. s_cached/s_chunk parameterized so a
    scaled-down variant can run under CoreSim."""
    n_sc = s_cached // s_chunk
    tpc = s_chunk // 128  # s-tiles per chunk
    assert s_cached % s_chunk == 0
    BW = 6  # s-tiles per exp block
    n_stiles = n_sc * tpc + 2  # cached + the two new s-tiles
    assert n_stiles % BW == 0, "block batching assumes BW | total s-tiles"
    n_blocks = n_stiles // BW
    n_acc = min(4, n_blocks)
    nc = bacc.Bacc("TRN2", target_bir_lowering=False, debug=False, num_devices=NCORES)

    xT = nc.declare_dram_parameter("xT", [128, NI, R], BF16, isOutput=False)
    wqT = nc.declare_dram_parameter("wqT", [128, NI, CD], BF16, isOutput=False)
    wkT = nc.declare_dram_parameter("wkT", [128, NI, CD], BF16, isOutput=False)
    wvT = nc.declare_dram_parameter("wvT", [128, NI, CD], BF16, isOutput=False)
    woT = nc.declare_dram_parameter("woT", [128, HL, DIM], BF16, isOutput=False)
    kTc = nc.declare_dram_parameter("kTc", [B, HL, HD, s_cached], BF16, isOutput=False)
    vc = nc.declare_dram_parameter(
        "vc", [B, HL, n_sc, 128, tpc, 128], BF16, isOutput=False
    )
    cosb = nc.declare_dram_parameter("cosb", [128, RC, CD // 2], BF16, isOutput=False)
    sinb = nc.declare_dram_parameter("sinb", [128, RC, CD // 2], BF16, isOutput=False)
    gq = nc.declare_dram_parameter("gq", [1, CD], F32, isOutput=False)
    gk = nc.declare_dram_parameter("gk", [1, CD], F32, isOutput=False)
    bq = nc.declare_dram_parameter("bq", [1, CD], F32, isOutput=False)
    bk = nc.declare_dram_parameter("bk", [1, CD], F32, isOutput=False)
    bv = nc.declare_dram_parameter("bv", [1, CD], F32, isOutput=False)
    out_d = nc.declare_dram_parameter("out", [R, DIM], BF16, isOutput=True)

    with tile.TileContext(nc) as tc:
        with (
            tc.tile_pool(name="const", bufs=1) as const,
            tc.tile_pool(name="dram", bufs=1, space="DRAM") as dram,
            tc.tile_pool(name="qkT", bufs=1) as pqkT,
            tc.tile_pool(name="vsb", bufs=1) as pvs,
            tc.tile_pool(name="attn", bufs=1) as pattn,
            tc.tile_pool(name="wo", bufs=1) as pwo,
            tc.tile_pool(name="kc", bufs=3) as pk,
            tc.tile_pool(name="vcp", bufs=3) as pvv,
        ):
            # ---- constants (DMAs deferred until after the first x/w groups) ----
            ident_bf = const.tile([128, 128], BF16)
            make_identity(nc, ident_bf)
            eps_t = const.tile([128, 1], F32)
            nc.vector.memset(eps_t, EPS)
            # dummy Sqrt pins its table load into idle scalar-engine time so
            # the rstd computation later doesn't pay it on the critical chain
            warm_t = const.tile([128, 1], F32)
            nc.scalar.activation(out=warm_t, in_=eps_t, func=AF.Sqrt)
            ones_row = const.tile([1, 128], BF16)
            nc.vector.memset(ones_row, 1.0)
            bv_row = const.tile([1, CD], F32)
            bvb = const.tile([1, CD], BF16)
            cos_t = const.tile([128, RC, CD // 2], BF16)
            sin_t = const.tile([128, RC, CD // 2], BF16)
            bcasts = {}
            for name in ("gq", "gk", "bq", "bk"):
                t = const.tile([128, CD], F32, tag=f"bc_{name}")
                bcasts[name] = t

            def load_bcast(name, src):
                nc.gpsimd.dma_start(
                    out=bcasts[name], in_=src[:].to_broadcast((128, CD))
                )

            def load_q_consts():
                # everything the Q epilogue reads (bias/gain/rope tables) must
                # be emitted before it, i.e. within the Q projection stream
                load_bcast("gq", gq)
                load_bcast("bq", bq)
                nc.sync.dma_start(out=cos_t, in_=cosb[:])
                nc.sync.dma_start(out=sin_t, in_=sinb[:])

            def load_consts():
                nc.sync.dma_start(out=bv_row, in_=bv[:])
                nc.vector.tensor_copy(out=bvb, in_=bv_row)
                load_bcast("gk", gk)
                load_bcast("bk", bk)

            # persistent activations
            q_kT = pqkT.tile([128, 2 * HL, R], BF16)  # [hd, 0:3 qheads | 3:6 kheads, r]
            vs = pvs.tile([128, RC, CD], BF16)  # new V natural
            attn_sb = pattn.tile([128, B * HL, S_NEW], BF16)  # normalized attn outT
            wo_sb = pwo.tile([128, HL, DIM], BF16)

            # K/V cache chunk streaming (pools persistent so the first chunks
            # can prefetch during the projections)
            chunk_order = [
                (b, h, sc) for b in range(B) for h in range(HL) for sc in range(n_sc)
            ]
            fetched = {}
            fetch_state = {"ptr": 0}

            def fetch_next():
                b, h, sc = chunk_order[fetch_state["ptr"]]
                fetch_state["ptr"] += 1
                kt = pk.tile([128, s_chunk], BF16, tag="kt")
                nc.sync.dma_start(out=kt, in_=kTc[b, h, :, ts(sc, s_chunk)])
                vt = pvv.tile([128, tpc, 128], BF16, tag="vt")
                nc.sync.dma_start(out=vt, in_=vc[b, h, sc])
                fetched[(b, h, sc)] = (kt, vt)
                # slip the (head-sliced) Wo load into the early attention
                # chunk slack; it is only needed once b0's out-proj starts
                p = fetch_state["ptr"]
                if 2 <= p <= 1 + HL:
                    nc.sync.dma_start(
                        out=wo_sb[:, p - 2, :], in_=woT[:, p - 2, :]
                    )

            with (
                tc.tile_pool(name="xt", bufs=6) as px,
                tc.tile_pool(name="wstream", bufs=3) as pw,
                tc.tile_pool(name="projps", bufs=4, space="PSUM") as pp,
                tc.tile_pool(name="qknat", bufs=1) as pqk,
                tc.tile_pool(name="pre", bufs=1) as ppre,
                tc.tile_pool(name="scratch", bufs=2) as scratch,
                tc.tile_pool(name="stats", bufs=1) as pstats,
                tc.tile_pool(name="tps", bufs=2, space="PSUM") as ptp,
            ):
                x_tiles = []

                qs = pqk.tile([128, RC, CD], F32, tag="qs")
                ks = pqk.tile([128, RC, CD], F32, tag="ks")
                preq = ppre.tile([128, RC, CD], BF16, tag="preq")
                prek = ppre.tile([128, RC, CD], BF16, tag="prek")
                ssq = pstats.tile([128, 8], F32, tag="ssq")
                ssq_red = pstats.tile([128, 8], F32, tag="ssq_red")
                rstd = pstats.tile([128, 8], F32, tag="rstd")

                def projection(wT_d, load_x=False, dma_hook=None, bias_mm=False):
                    """bias_mm: close the accumulation with a rank-1
                    ones x bias matmul instead of stopping at the last
                    contraction chunk (folds the bias add into PE)."""
                    psums = [
                        pp.tile([128, CD], F32, name="projps", tag="projps")
                        for _ in range(RC)
                    ]
                    for g in range(NG):
                        if load_x:
                            x_t = px.tile([128, G, R], BF16, tag="xg")
                            if g == 0:
                                # halved first transfers so the very first
                                # matmul can issue ~1.3us earlier
                                nc.sync.dma_start(
                                    out=x_t[:, 0 : G // 2, :],
                                    in_=xT[:, 0 : G // 2, :],
                                )
                            else:
                                nc.sync.dma_start(out=x_t, in_=xT[:, ts(g, G), :])
                            x_tiles.append(x_t)
                        w_t = pw.tile([128, G, CD], BF16, tag="wg")
                        if g == 0:
                            nc.sync.dma_start(
                                out=w_t[:, 0 : G // 2, :], in_=wT_d[:, 0 : G // 2, :]
                            )
                            if load_x:
                                nc.sync.dma_start(
                                    out=x_tiles[0][:, G // 2 : G, :],
                                    in_=xT[:, G // 2 : G, :],
                                )
                            nc.sync.dma_start(
                                out=w_t[:, G // 2 : G, :],
                                in_=wT_d[:, G // 2 : G, :],
                            )
                        else:
                            nc.sync.dma_start(out=w_t, in_=wT_d[:, ts(g, G), :])
                        if dma_hook is not None:
                            dma_hook(g)
                        for ii in range(G):
                            i = g * G + ii
                            for rc in range(RC):
                                nc.tensor.matmul(
                                    out=psums[rc],
                                    lhsT=x_tiles[g][:, ii, ts(rc, 128)],
                                    rhs=w_t[:, ii, :],
                                    start=(i == 0),
                                    stop=(i == NI - 1 and not bias_mm),
                                )
                    if bias_mm:
                        for rc in range(RC):
                            nc.tensor.matmul(
                                out=psums[rc],
                                lhsT=ones_row[:],
                                rhs=bvb[:],
                                start=False,
                                stop=True,
                            )
                    return psums

                def qk_epilogue(psums, nat, pre, gname, bname, ssq_col):
                    # bias-adds first so the PSUM banks free for the next
                    # projection, then the ssq partials (they gate the
                    # AllReduce), then the gain/rope tail
                    for rc in range(RC):
                        nc.vector.tensor_add(
                            out=nat[:, rc, :], in0=psums[rc], in1=bcasts[bname]
                        )
                    for rc in range(RC):
                        # (tensor_tensor_reduce wedges the device; use
                        # square + reduce_sum instead)
                        sq = scratch.tile([128, CD], F32, tag="sq")
                        nc.vector.tensor_mul(
                            out=sq, in0=nat[:, rc, :], in1=nat[:, rc, :]
                        )
                        nc.vector.reduce_sum(
                            out=ssq[:, ssq_col + rc : ssq_col + rc + 1],
                            in_=sq[:],
                            axis=mybir.AxisListType.X,
                        )
                    for rc in range(RC):
                        gsc = scratch.tile([128, CD], F32, tag="gsc")
                        nc.vector.tensor_mul(
                            out=gsc, in0=nat[:, rc, :], in1=bcasts[gname]
                        )
                        # rope into the bf16 pre-transpose tile
                        gp = gsc.rearrange("p (j two) -> p j two", two=2)
                        rp = pre[:, rc, :].rearrange("p (j two) -> p j two", two=2)
                        ce = cos_t[:, rc, :]
                        se = sin_t[:, rc, :]
                        t1 = scratch.tile([128, CD // 2], F32, tag="t1")
                        t2 = scratch.tile([128, CD // 2], F32, tag="t2")
                        nc.vector.tensor_mul(out=t1, in0=gp[:, :, 0], in1=ce)
                        nc.vector.tensor_mul(out=t2, in0=gp[:, :, 1], in1=se)
                        nc.vector.tensor_sub(out=rp[:, :, 0], in0=t1, in1=t2)
                        t3 = scratch.tile([128, CD // 2], F32, tag="t3")
                        t4 = scratch.tile([128, CD // 2], F32, tag="t4")
                        nc.vector.tensor_mul(out=t3, in0=gp[:, :, 0], in1=se)
                        nc.vector.tensor_mul(out=t4, in0=gp[:, :, 1], in1=ce)
                        nc.vector.tensor_add(out=rp[:, :, 1], in0=t3, in1=t4)

                def q_hook(g):
                    if g == 5:
                        load_q_consts()

                def k_hook(g):
                    # consts (cos/sin/gains/biases) are first read by the Q
                    # epilogue, which runs during K-proj — loading them here
                    # keeps the Q-phase DMA window (which paces the PE start)
                    # clear for x and Wq
                    if g == 0:
                        load_consts()
                    elif g == 5:
                        fetch_next()

                qps = projection(wqT, load_x=True, dma_hook=q_hook)
                qk_epilogue(qps, qs, preq, "gq", "bq", 0)
                kps = projection(wkT, dma_hook=k_hook)
                qk_epilogue(kps, ks, prek, "gk", "bk", 4)

                # tiny AllReduce of the norm statistics
                cc_in = dram.tile([128, 8], F32)
                cc_out = dram.tile([128, 8], F32)
                nc.sync.dma_start(out=cc_in[:], in_=ssq)
                if collective:
                    nc.gpsimd.collective_compute(
                        "AllReduce",
                        OP.add,
                        replica_groups=[list(range(NCORES))],
                        ins=[cc_in.opt()],
                        outs=[cc_out.opt()],
                    )
                else:
                    nc.sync.dma_start(out=cc_out[:], in_=cc_in[:])
                nc.sync.dma_start(out=ssq_red, in_=cc_out[:])

                # V projection + second K/V chunk prefetch fill the AllReduce
                # latency window
                def v_hook(g):
                    if g == 3 and len(chunk_order) > 1:
                        fetch_next()

                vps = projection(wvT, dma_hook=v_hook, bias_mm=True)

                # rstd = 1/sqrt(ssq/DIM + eps)
                nc.scalar.activation(
                    out=rstd, in_=ssq_red, func=AF.Sqrt, bias=eps_t, scale=1.0 / DIM
                )
                nc.vector.reciprocal(out=rstd, in_=rstd)
                # re-warm the Exp table set (Sqrt evicted it); reading rstd
                # pins this after the Sqrt in schedule order, so the 1.3us
                # load overlaps the transposes instead of the first score exp
                nc.scalar.activation(out=warm_t, in_=rstd[:, 0:1], func=AF.Exp)
                # V epilogue is a plain PSUM->SBUF copy (the bias was folded
                # into the projection as a rank-1 matmul); it rides on ACT
                # where it can't disturb the DVE recip/scale chain or the
                # Pool transpose copies
                for rc in range(RC):
                    nc.scalar.copy(out=vs[:, rc, :], in_=vps[rc])

                for pre, col0 in ((preq, 0), (prek, 4)):
                    for rc in range(RC):
                        nc.vector.tensor_scalar_mul(
                            out=pre[:, rc, :],
                            in0=pre[:, rc, :],
                            scalar1=rstd[:, col0 + rc : col0 + rc + 1],
                        )

                # head transposes: the 4 row-chunk transposes of a head share
                # one PSUM bank, so each head drains with a single [128,512]
                # DVE copy (6 copies total instead of 24) — this is what keeps
                # the projection->attention PSUM-pool handoff short
                for pre, base in ((preq, 0), (prek, HL)):
                    for h in range(HL):
                        ptg = ptp.tile([128, RC, 128], BF16, name="ptg")
                        for rc in range(RC):
                            nc.tensor.transpose(
                                out=ptg[:, rc, :],
                                in_=pre[:, rc, ts(h, 128)],
                                identity=ident_bf[:],
                            )
                        nc.vector.tensor_copy(out=q_kT[:, base + h, :], in_=ptg[:])

            # ---- attention ----
            # psc's [128,1536] ring is shared by the score blocks AND the
            # out-proj accumulators (interleaved lifetimes), which is what
            # lets the score pipeline run 2 blocks deep in 8 PSUM banks.
            with (
                tc.tile_pool(name="scoreps", bufs=2, space="PSUM") as psc,
                tc.tile_pool(name="outps", bufs=1, space="PSUM") as pout,
                tc.tile_pool(name="opj", bufs=1, space="PSUM") as popj,
                tc.tile_pool(name="ptiles", bufs=8) as ppb,
                tc.tile_pool(name="accs", bufs=2 * n_acc) as pacc,
                tc.tile_pool(name="small", bufs=2) as psm,
                tc.tile_pool(name="outsb", bufs=2) as pos,
            ):
                out_ps = pout.tile([128, 2 * S_NEW], F32)  # two heads in flight

                # out-proj chunk emitters; b0's chunks are deferred and
                # trickled into b1's (ACT-bound) attention stream so the PE
                # slack absorbs them instead of the normalize tail
                outproj_pending = []
                outproj_seq = {"n": 0, "mode": "trickle"}
                fin_tiles = {}

                def make_outproj_chunks(b):
                    chunks = []
                    for rh in range(2):
                        out_sb = pos.tile([128, DIM], BF16, name="out_sb", tag="osb")
                        r0 = b * S_NEW + rh * 128
                        for oc in range(DIM // 512):

                            def emit(b=b, rh=rh, oc=oc, out_sb=out_sb, r0=r0):
                                # trickled chunks take the dedicated bank so
                                # they can't flip the score ring's parity
                                # (which would serialize a whole exp); end
                                # chunks pipeline on the then-free score ring
                                if outproj_seq["mode"] == "trickle":
                                    o_ps = popj.tile(
                                        [128, 512], F32, name="o_ps", tag="opj"
                                    )
                                else:
                                    o_ps = psc.tile(
                                        [128, BW * S_NEW], F32, name="o_ps",
                                        tag="sblk",
                                    )
                                for hh in range(HL):
                                    nc.tensor.matmul(
                                        out=o_ps[:, 0:512],
                                        lhsT=attn_sb[:, b * HL + hh, ts(rh, 128)],
                                        rhs=wo_sb[:, hh, ts(oc, 512)],
                                        start=(hh == 0),
                                        stop=(hh == HL - 1),
                                    )
                                # copies alternate DVE/ACT (Pool cannot read
                                # PSUM — walrus rejects it)
                                if outproj_seq["n"] % 2 == 0:
                                    nc.vector.tensor_copy(
                                        out=out_sb[:, ts(oc, 512)], in_=o_ps[:, 0:512]
                                    )
                                else:
                                    nc.scalar.copy(
                                        out=out_sb[:, ts(oc, 512)], in_=o_ps[:, 0:512]
                                    )
                                outproj_seq["n"] += 1
                                if outproj_seq["mode"] == "drain":
                                    # finest-grained writeout at the end so the
                                    # last DMA starts right after the last copy
                                    nc.sync.dma_start(
                                        out=out_d[r0 : r0 + 128, ts(oc, 512)],
                                        in_=out_sb[:, ts(oc, 512)],
                                    )
                                elif oc % 2 == 1:
                                    c0 = (oc - 1) * 512
                                    nc.sync.dma_start(
                                        out=out_d[r0 : r0 + 128, c0 : c0 + 1024],
                                        in_=out_sb[:, c0 : c0 + 1024],
                                    )

                            chunks.append(emit)
                    return chunks

                # the AV/score pipeline spans head boundaries: each pend entry
                # carries its head's PSUM half and start/stop flags, and the
                # denominator finalize for a head is emitted when its last AV
                # group flushes (a couple of blocks into the next head), so PE
                # never drains between heads
                pend = []

                def flush_oldest():
                    e = pend.pop(0)
                    for j in range(BW):
                        nc.tensor.matmul(
                            out=e["half"],
                            lhsT=e["vsl"][j],
                            rhs=e["p_t"][:, ts(j, S_NEW)],
                            start=(e["first"] and j == 0),
                            stop=(e["last"] and j == BW - 1),
                        )
                    if e["last"]:
                        e["fin"]()

                for b in range(B):
                    for h in range(HL):
                        bh = b * HL + h
                        qT_bh = q_kT[:, h, b * S_NEW : (b + 1) * S_NEW]
                        out_half = out_ps[:, (bh % 2) * S_NEW : (bh % 2 + 1) * S_NEW]
                        accs = [
                            pacc.tile([128, BW * S_NEW], BF16, name="acc", tag="acc")
                            for _ in range(n_acc)
                        ]
                        state = {"bi": 0}
                        # accumulator assignment: the LAST block must land in
                        # acc0 so every cross-accumulator fold can be emitted
                        # before the final block, leaving only
                        # acc0+=acc1 -> slot folds on the per-head tail
                        if n_acc == 4:
                            shift = (n_blocks - 1) % 4
                            macc = [(j - shift) % 4 for j in range(4)]
                        else:
                            macc = list(range(max(n_acc, 1)))
                        amap = lambda bi: macc[bi % n_acc]
                        last_blk = [
                            max((q for q in range(n_blocks) if amap(q) == j),
                                default=-1)
                            for j in range(n_acc)
                        ]
                        fold_at = (
                            max(last_blk[1], last_blk[2], last_blk[3])
                            if n_acc == 4 else -1
                        )

                        def finalize(bh=bh, accs=accs, out_half=out_half):
                            # fold bf16 accumulators, reduce across partitions
                            # on gpsimd, reciprocal, normalize
                            a0 = accs[0]
                            if n_acc == 4:
                                # accs 2 and 3 were folded into acc1 at
                                # fold_at; only one cross-fold remains here
                                nc.vector.tensor_add(out=a0, in0=a0, in1=accs[1])
                            else:
                                if n_acc > 1:
                                    nc.vector.tensor_add(
                                        out=a0, in0=a0, in1=accs[1]
                                    )
                                if n_acc > 2:
                                    nc.vector.tensor_add(
                                        out=a0, in0=a0, in1=accs[2]
                                    )
                            half = BW * S_NEW // 2
                            f768 = psm.tile([128, half], BF16, tag="f768")
                            nc.vector.tensor_add(
                                out=f768, in0=a0[:, 0:half], in1=a0[:, half : 2 * half]
                            )
                            f256 = psm.tile([128, 256], BF16, tag="f256")
                            nc.vector.tensor_add(
                                out=f256, in0=f768[:, 0:256], in1=f768[:, 256:512]
                            )
                            if half > 512:
                                nc.vector.tensor_add(
                                    out=f256, in0=f256, in1=f768[:, 512:768]
                                )
                            dsum = psm.tile([128, 256], F32, tag="dsum")
                            nc.gpsimd.partition_all_reduce(
                                dsum[:], f256[:], 128, bass_isa.ReduceOp.add
                            )
                            rec = psm.tile([128, 256], F32, tag="rec")
                            nc.vector.reciprocal(out=rec, in_=dsum)
                            nc.vector.tensor_mul(
                                out=attn_sb[:, bh, :], in0=out_half, in1=rec
                            )
                            fin_tiles.update(a0=a0, f768=f768, f256=f256, rec=rec)

                        def do_block(ksl, vsl):
                            s_t = psc.tile([128, BW * S_NEW], F32, tag="sblk")
                            for j in range(BW):
                                nc.tensor.matmul(
                                    out=s_t[:, ts(j, S_NEW)],
                                    lhsT=ksl[j],
                                    rhs=qT_bh,
                                    start=True,
                                    stop=True,
                                )
                            if len(pend) == 2:
                                flush_oldest()
                            p_t = ppb.tile([128, BW * S_NEW], BF16, tag="pblk")
                            nc.scalar.activation(
                                out=p_t, in_=s_t, func=AF.Exp, scale=SCALE
                            )
                            bi = state["bi"]
                            ai = amap(bi)
                            if bi < n_acc:
                                nc.vector.tensor_copy(out=accs[ai], in_=p_t)
                            else:
                                a = accs[ai]
                                nc.vector.tensor_add(out=a, in0=a, in1=p_t)
                            if n_acc == 4 and bi == fold_at:
                                nc.vector.tensor_add(
                                    out=accs[1], in0=accs[1], in1=accs[2]
                                )
                                nc.vector.tensor_add(
                                    out=accs[1], in0=accs[1], in1=accs[3]
                                )
                            state["bi"] = bi + 1
                            pend.append(
                                {
                                    "vsl": vsl,
                                    "p_t": p_t,
                                    "half": out_half,
                                    "first": bi == 0,
                                    "last": bi == n_blocks - 1,
                                    "fin": finalize,
                                }
                            )
                            # trickle one deferred out-proj chunk every 2nd
                            # block into the PE stream
                            if outproj_pending and state["bi"] % 2 == 0:
                                outproj_pending.pop(0)()

                        def tile_stream():
                            for sc in range(n_sc):
                                kt, vt = fetched.pop((b, h, sc))
                                if fetch_state["ptr"] < len(chunk_order):
                                    fetch_next()
                                for t in range(tpc):
                                    yield kt[:, ts(t, 128)], vt[:, t, :]
                            for j in range(2):
                                yield (
                                    q_kT[
                                        :, HL + h,
                                        b * S_NEW + 128 * j : b * S_NEW + 128 * (j + 1),
                                    ],
                                    vs[:, b * 2 + j, ts(h, 128)],
                                )

                        stream = tile_stream()
                        for _ in range(n_blocks):
                            pairs = [next(stream) for _ in range(BW)]
                            do_block([p[0] for p in pairs], [p[1] for p in pairs])

                    # defer b0's out-proj into b1's attention; drain the
                    # pipeline and emit the remaining chunks at the very end
                    outproj_pending.extend(make_outproj_chunks(b))
                    if b == B - 1:
                        while pend:
                            flush_oldest()
                        # PE heaters: trivial matmuls gated on the last head's
                        # denominator chain keep the PE clock warm through the
                        # ~4us normalize window, so the final out-proj doesn't
                        # run at the cold/mid clock
                        if n_blocks >= 4:
                            for key, wdt in (
                                ("a0", 512), ("f768", 512), ("f256", 256),
                            ):
                                src = fin_tiles[key]
                                hp = popj.tile([128, 512], F32, name="hp", tag="opj")
                                nc.tensor.matmul(
                                    out=hp[0:1, 0:wdt],
                                    lhsT=ones_row[0:1, 0:1],
                                    rhs=src[0:1, 0:wdt],
                                    start=True,
                                    stop=True,
                                )
                            hp = popj.tile([128, 512], F32, name="hp", tag="opj")
                            nc.tensor.matmul(
                                out=hp[0:1, 0:256],
                                lhsT=eps_t[0:1, 0:1],
                                rhs=fin_tiles["rec"][0:1, 0:256],
                                start=True,
                                stop=True,
                            )
                        outproj_seq["mode"] = "drain"
                        for emit in outproj_pending:
                            emit()
                        outproj_pending.clear()

    nc.compile()
    return nc


_CACHE = {}


def _get_nc(s_cached, s_chunk):
    key = (s_cached, s_chunk)
    if key not in _CACHE:
        _CACHE[key] = build(s_cached, s_chunk)
    return _CACHE[key]


def make_in_maps(x, freqs, k_cache, v_cache, Wq, bq, Wk, bk, Wv, bv, Wo, bo, gq, gk,
                 s_chunk=4096):
    s_cached = k_cache.shape[1]
    n_sc = s_cached // s_chunk
    tpc = s_chunk // 128
    x2 = np.ascontiguousarray(x, dtype=np.float32).reshape(R, DIM)
    # [128, NI, R] with element (p, n, r) = xT[n*128+p, r] = x2[r, n*128+p]
    xT = np.ascontiguousarray(
        x2.T.reshape(NI, 128, R).transpose(1, 0, 2).astype(ml_dtypes.bfloat16)
    )
    cos = np.cos(np.asarray(freqs, dtype=np.float32))
    sin = np.sin(np.asarray(freqs, dtype=np.float32))

    def prearrange_rot(t):
        # [S_new, 64] -> [R, 192] (b-tile, head-tile) -> [128, RC, 192]
        full = np.tile(np.tile(t, (B, 1)), (1, HL))
        return np.ascontiguousarray(
            full.reshape(RC, 128, CD // 2).transpose(1, 0, 2)
            .astype(ml_dtypes.bfloat16)
        )

    cosb = prearrange_rot(cos)
    sinb = prearrange_rot(sin)
    Wq = np.asarray(Wq, dtype=np.float32)
    Wk = np.asarray(Wk, dtype=np.float32)
    Wv = np.asarray(Wv, dtype=np.float32)
    Wo = np.asarray(Wo, dtype=np.float32)
    k_cache = np.asarray(k_cache, dtype=np.float32)
    v_cache = np.asarray(v_cache, dtype=np.float32)

    def prew(Wslice):
        # W[c_slice, :].T = [DIM, CD] -> [128, NI, CD]
        return np.ascontiguousarray(
            Wslice.T.reshape(NI, 128, CD).transpose(1, 0, 2).astype(ml_dtypes.bfloat16)
        )

    in_maps = []
    for c in range(NCORES):
        cs, ce = c * CD, (c + 1) * CD
        kTc = np.ascontiguousarray(
            k_cache[:, :, cs:ce]
            .reshape(B, s_cached, HL, HD)
            .transpose(0, 2, 3, 1)
            .astype(ml_dtypes.bfloat16)
        )
        # [B, HL, n_sc, 128, tpc, 128]: (b,h,sc,p,t,d) = v[b, sc*s_chunk+t*128+p, cs+h*128+d]
        vcc = np.ascontiguousarray(
            v_cache[:, :, cs:ce]
            .reshape(B, n_sc, tpc, 128, HL, 128)
            .transpose(0, 4, 1, 3, 2, 5)
            .astype(ml_dtypes.bfloat16)
        )
        woT = np.ascontiguousarray(
            Wo[:, cs:ce].T.reshape(HL, 128, DIM).transpose(1, 0, 2)
            .astype(ml_dtypes.bfloat16)
        )
        in_maps.append(
            {
                "xT": xT,
                "wqT": prew(Wq[cs:ce, :]),
                "wkT": prew(Wk[cs:ce, :]),
                "wvT": prew(Wv[cs:ce, :]),
                "woT": woT,
                "kTc": kTc,
                "vc": vcc,
                "cosb": cosb,
                "sinb": sinb,
                "gq": np.ascontiguousarray(gq[cs:ce])[None, :].astype(np.float32),
                "gk": np.ascontiguousarray(gk[cs:ce])[None, :].astype(np.float32),
                "bq": np.ascontiguousarray(bq[cs:ce])[None, :].astype(np.float32),
                "bk": np.ascontiguousarray(bk[cs:ce])[None, :].astype(np.float32),
                "bv": np.ascontiguousarray(bv[cs:ce])[None, :].astype(np.float32),
            }
        )
    return in_maps


def kernel(x, freqs, k_cache, v_cache, Wq, bq, Wk, bk, Wv, bv, Wo, bo, gq, gk):
    s_cached = k_cache.shape[1]
    s_chunk = 4096 if s_cached % 4096 == 0 else 512
    nc = _get_nc(s_cached, s_chunk)
    in_maps = make_in_maps(
        x, freqs, k_cache, v_cache, Wq, bq, Wk, bk, Wv, bv, Wo, bo, gq, gk,
        s_chunk=s_chunk,
    )
    res = run_bass_kernel_spmd(nc, in_maps, list(range(NCORES)))
    acc = np.zeros((R, DIM), dtype=np.float64)
    for c in range(NCORES):
        acc += res.results[c]["out"].astype(np.float64)
    out = (acc + np.asarray(bo, dtype=np.float64)[None, :]).astype(np.float32)
    return out.reshape(B, S_NEW, DIM)
